# revision 58
# baseline (speedup 1.0000x reference)
"""BinaryTreeCRF inside-algorithm kernel for 8 Trainium2 NeuronCores (v2).

Strategy (hardcoded for hidden=[16383,1024], L=32, depth 13):
  - Big tree cut at level 3: core c owns the subtree rooted at heap node 7+c.
  - Device does the bottom two combine levels (1024 leaves -> 512 -> 256
    nodes, 3 passes of 256 parents); host finishes levels 3..10 + the top
    3 big-tree levels in float64 (it also computes the emission rows for
    those 255+7 nodes itself, so their hidden rows never ship to devices).
  - Shipped columns (1792/core) are pass-major 256-col slices
    [rl0|rr0|elev0|rl1|rr1|elev1|elev2] so each combine pass depends on a
    prefix of the (FIFO-ordered) slice DMAs.
  - E is kept as 4 row-strips of partial sums ([128,n]: strip g rows hold
    chunks {g,g+4} of the K=1024 contraction) -> the E matmul uses all 128
    PE columns via 4x col-tiling; bias/4 is folded into each strip by the
    PSUM->SBUF copy. Strips are reduced for free inside the combine's
    replicate matmul (R1 selector).
  - Combine pass: stk2 = [rl;rr;rl;rr] actual values via 4 col-tiled
    replicate matmuls; logP_c = sel2_c^T @ stk (K=64, one MM per chunk,
    2x row-tiled); P = exp(logP) (ACT, no mean-centering needed in f32);
    S = texp^T @ P (8 accum MMs); resid = elev + ln S added in place into
    the elev strip block.
"""

import numpy as np
import ml_dtypes

BF16 = ml_dtypes.bfloat16

INPUT_SIZE = 1024
L = 32
DEPTH = 13
N_CORES = 8
NSLICE = 4
SL = 256
COLS = NSLICE * SL  # 1024


def _bitrev(x, bits):
    x = np.asarray(x, dtype=np.int64)
    out = np.zeros_like(x)
    for i in range(bits):
        out = (out << 1) | ((x >> i) & 1)
    return out


def _level_cols(c, lev):
    """Heap index per column q of the (bit-reversed) level-`lev` block."""
    m = 1 << (10 - lev)
    d = DEPTH - lev
    q = np.arange(m)
    j = _bitrev(q, 10 - lev)
    return (1 << d) - 1 + c * m + j


def _slices_for_core(c):
    lv0 = _level_cols(c, 0)
    return [lv0[0:256], lv0[512:768], lv0[256:512], lv0[768:1024]]


_NC = None


def _build_bass():
    global _NC
    if _NC is not None:
        return _NC
    from concourse import bacc, mybir
    from concourse.tile import TileContext
    from concourse.tile_rust import add_dep_helper

    dtb = mybir.dt.bfloat16
    dtf = mybir.dt.float32
    AF = mybir.ActivationFunctionType

    # Row-tiled logP (tile_position (64,0) K=64 matmuls) compiles and passes
    # CoreSim but faults on hardware — keep the flat variant.
    import os
    flat_logp = os.environ.get("V2_FLAT_LOGP", "1") == "1"

    dt8 = mybir.dt.float8e4

    nc = bacc.Bacc()
    hsB = nc.dram_tensor("hsB", [128, NSLICE, 8, SL], dt8, kind="ExternalInput")
    wpk = nc.dram_tensor("wpack", [128, 256], dt8, kind="ExternalInput")
    bpk = nc.dram_tensor("bpack", [128, 2], dtb, kind="ExternalInput")
    cpk = nc.dram_tensor("cpack2", [128, 1312], dtb, kind="ExternalInput")
    outR = nc.dram_tensor("outResid", [L, 2 * SL], dtb, kind="ExternalOutput")

    with TileContext(nc) as tc:
        with tc.tile_pool(name="consts", bufs=1) as consts, \
             tc.tile_pool(name="hs", bufs=1) as hpool, \
             tc.tile_pool(name="state", bufs=1) as state, \
             tc.tile_pool(name="stksb", bufs=2) as stksb, \
             tc.tile_pool(name="pbuf", bufs=2) as pbuf, \
             tc.tile_pool(name="tmp", bufs=4) as tmp, \
             tc.tile_pool(name="pslogp", bufs=1, space="PSUM") as pslogp, \
             tc.tile_pool(name="pse", bufs=2, space="PSUM") as pse, \
             tc.tile_pool(name="psstk", bufs=1, space="PSUM") as psstk, \
             tc.tile_pool(name="pss", bufs=1, space="PSUM") as pss:

            bp = consts.tile([128, 2], dtb, tag="bpack")
            nc.sync.dma_start(out=bp, in_=bpk[:, :])
            wp = consts.tile([128, 256], dt8, tag="wpack")
            wp_dma = nc.sync.dma_start(out=wp, in_=wpk[:, :])
            wp_dma = wp_dma.ins if hasattr(wp_dma, "ins") else wp_dma

            cp = consts.tile([128, 1312], dtb, tag="cpack2")
            cp_dma = nc.scalar.dma_start(out=cp, in_=cpk[:, :])
            cp_dma = cp_dma.ins if hasattr(cp_dma, "ins") else cp_dma

            R1_t = cp[:, 0:32]
            sel2_t = cp[:, 32:1056]      # chunk c at cols 32+128c; rows 0:64
                                         # (odd chunks also mirrored at 64:128)
            texp_t = cp[:, 1056:1312]

            # No inter-DMA deps: all on the one sync HWDGE ring, which
            # drains FIFO — serializing transfers without serializing the
            # ~2us completion receipts (dep-chaining cost ~3us per slice).
            hs_all = hpool.tile([128, NSLICE, 8, SL], dt8, tag="hs")
            for s in range(NSLICE):
                nc.sync.dma_start(out=hs_all[:, s, :, :], in_=hsB[:, s, :, :])

            E_bf = state.tile([128, COLS], dtb, tag="E_bf")

            # bias/4 per strip row, upcast to f32 (also triggers the ACT
            # table load early, off the critical path)
            bq_f = tmp.tile([128, 1], dtf, tag="bq_f")
            nc.scalar.activation(out=bq_f, in_=bp[:, 0:1], func=AF.Identity)

            # PE warm-up on a zeroed SBUF tile: no DMA dependency, so the
            # warm-up runs during the load phase (not after it, which would
            # push the first real matmul out by its whole duration) and the
            # HAM un-throttles before the first E matmul.
            jnkT = tmp.tile([128, SL], dtb, tag="jnkT")
            nc.gpsimd.memset(jnkT, 0.0)
            jnk = pse.tile([128, SL], dtf, tag="psE")
            for _ in range(14):
                nc.tensor.matmul(jnk, lhsT=jnkT[:, 0:128], rhs=jnkT,
                                 start=True, stop=True)

            def emit_E(s):
                psE = pse.tile([128, SL], dtf, tag="psE")
                for g in range(4):
                    for r in range(2):
                        ch = g + 4 * r
                        nc.tensor.matmul(
                            psE[32 * g:32 * g + 32, :],
                            lhsT=wp[:, 32 * ch:32 * ch + 32],
                            rhs=hs_all[:, s, ch, :],
                            start=(r == 0), stop=(r == 1),
                            tile_position=(0, 32 * g))
                # psE holds 16*E (W shipped as 16W for fp8 range): scale
                # back and add bias/4 in the one PSUM->SBUF copy
                nc.vector.tensor_scalar(
                    out=E_bf[:, s * SL:(s + 1) * SL], in0=psE,
                    scalar1=1.0 / 16.0, scalar2=bq_f,
                    op0=mybir.AluOpType.mult, op1=mybir.AluOpType.add)

            def combine_rep_logp(brl, brr):
                """Replicate+reduce strips, then the 8 K=64 logP matmuls.
                Full 256-parent passes: the logP matmuls are LDWEIGHTS-bound
                (P=128 selector loads), so N=256 keeps LDW:MM at 1:1."""
                stk2 = psstk.tile([128, SL], dtf, tag="stk2")
                for i, bsrc in enumerate((brl, brr, brl, brr)):
                    nc.tensor.matmul(
                        stk2[32 * i:32 * i + 32, :], lhsT=R1_t,
                        rhs=E_bf[:, bsrc:bsrc + SL],
                        start=True, stop=True, tile_position=(0, 32 * i))
                stk_sb = stksb.tile([128, SL], dtb, tag="stk_sb")
                nc.vector.tensor_copy(stk_sb, stk2)
                # Two separate PSUM tiles so each exp depends on only its 4
                # matmuls (Tile tracks deps at tile granularity): exp of the
                # first half overlaps the second half's matmuls.
                P = pbuf.tile([128, 8, SL], dtb, tag="P")
                for t, tag in ((0, "logP"), (1, "logPb")):
                    logP = pslogp.tile([128, 4, SL], dtf, tag=tag)
                    for c4 in range(4):
                        ch = 4 * t + c4
                        half = 0 if flat_logp else 64 * (ch & 1)
                        nc.tensor.matmul(
                            logP[:, c4, :],
                            lhsT=sel2_t[half:half + 64,
                                        128 * ch:128 * ch + 128],
                            rhs=stk_sb[half:half + 64, :],
                            start=True, stop=True, tile_position=(half, 0))
                    nc.scalar.activation(out=P[:, 4 * t:4 * t + 4, :],
                                         in_=logP, func=AF.Exp)
                return P

            lnS_out = state.tile([L, 2, SL], dtb, tag="lnS_out")

            def combine_S(P, pp):
                S = pss.tile([32, SL], dtf, tag="sps")
                for ch in range(8):
                    nc.tensor.matmul(
                        S, lhsT=texp_t[:, 32 * ch:32 * ch + 32],
                        rhs=P[:, ch, :], start=(ch == 0), stop=(ch == 7))
                nc.scalar.activation(out=lnS_out[:, pp, :], in_=S, func=AF.Ln)

            # pipeline: E slices chase the DMAs; combine passes chase E.
            # Device does only the leaf level and ships lnS: the host's f64
            # finisher adds its own emission rows for level 1 and up, so no
            # elev columns ever ship to the device and there is no resid-add
            # dependency on the kernel tail.
            #
            # Both passes' rep/logP matmuls are emitted before either S
            # group: the S matmuls only wait on exp, while pass 1's logP
            # needs just E2/E3 — front-loading it lets the two passes' exps
            # run back-to-back on the ACT engine.
            emit_E(0)
            emit_E(1)
            P0 = combine_rep_logp(0, 256)
            emit_E(2)
            emit_E(3)
            P1 = combine_rep_logp(512, 768)
            combine_S(P0, 0)
            combine_S(P1, 1)

            nc.sync.dma_start(out=outR[:, :], in_=lnS_out)

    # Pin Exp/Ln/Identity to the one table set containing all three, so the
    # ACT engine loads its function table exactly once.
    import concourse.bacc as _bacc_mod
    from concourse.hw_specs import get_activation_tables as _gat
    _keep = "natural_log_exp_and_others"
    _pin = {AF.Exp, AF.Ln, AF.Identity, AF.Copy}

    def _gat_pinned(arch):
        t = _gat(arch)
        return {name: (funcs if name == _keep else (set(funcs) - _pin))
                for name, funcs in t.items()}

    _orig_gat = _bacc_mod.get_activation_tables
    _bacc_mod.get_activation_tables = _gat_pinned
    try:
        nc.compile()
    finally:
        _bacc_mod.get_activation_tables = _orig_gat
    _NC = nc
    return nc


def _patch_light_tail():
    """Sem-only end-of-kernel barriers (the default drain + two full
    all-engine barriers cost ~9us of kernel tail)."""
    from concourse import tile as _tile_mod
    from concourse.vector_clock import ScopedClock

    def _dab_light(self, tick_clock, wait_clock):
        drain_inst = self.nc.sync.drain()
        wait_clock.add_sem_waits(
            drain_inst.ins, ScopedClock({None: tick_clock.global_clock})
        )
        self.nc.all_engine_barrier(sem_only=True)
        popped = self.nc._tile_sem_poison_stack.pop()
        assert popped is self._sem_poison
        # No end-of-kernel sem clears: the Bass preamble range-clears every
        # kernel semaphore at the next launch, so the teardown is redundant.

    _tile_mod.TileContext._drain_and_barrier = _dab_light


_patch_light_tail()


def _prep_in_maps(hidden, W, b, trans):
    """Build per-core input dicts (host-side shard/transpose/cast)."""
    import ml_dtypes as _mld
    FP8 = _mld.float8_e4m3
    # wpack: E-matmul lhsT chunks, shipped as 16*W so the fp8 values sit in
    # e4m3's normal range (W ~ N(0, 1/32^2)); the device scales E back.
    wpack = np.zeros((128, 256), dtype=FP8)
    for ch in range(8):
        # wTr4[p, 32ch+m] = 16 * W[m, 128ch+p]
        wpack[:, 32 * ch:32 * ch + 32] = \
            (16.0 * W[:, 128 * ch:128 * ch + 128].T).astype(FP8)
    bpack = np.zeros((128, 2), dtype=BF16)
    bpack[:, 0] = np.tile(b.astype(np.float64) / 4.0, 4).astype(BF16)

    cpack2 = np.zeros((128, 1312), dtype=np.float32)
    for g in range(4):
        cpack2[32 * g:32 * g + 32, 0:32] = np.eye(32)
    for ch in range(8):
        blk = 32 + 128 * ch
        for p in range(128):
            # selector at rows 0:64 (all chunks) and mirrored at 64:128
            # (for the row-tiled variant's odd chunks)
            for rows in (0, 64):
                cpack2[rows + 4 * ch + p // 32, blk + p] += 1.0
                cpack2[rows + 32 + p % 32, blk + p] += 1.0
    Texp = np.exp(trans.astype(np.float64)).astype(np.float32)  # [k,l,r]
    for ch in range(8):
        for p in range(128):
            cpack2[p, 1056 + 32 * ch:1056 + 32 * ch + 32] = \
                Texp[:, 4 * ch + p // 32, p % 32]
    cpack2 = cpack2.astype(BF16)

    in_maps = []
    for c in range(N_CORES):
        cols = np.concatenate(_slices_for_core(c))
        hsc = hidden[cols].astype(FP8)                        # [1792, 1024]
        # hs[p, s, ch, j] = h[col s*256+j, 128ch+p]
        hs = np.ascontiguousarray(
            hsc.reshape(NSLICE, SL, 8, 128).transpose(3, 0, 2, 1))
        in_maps.append({"hsB": hs, "wpack": wpack, "bpack": bpack,
                        "cpack2": cpack2})
    return in_maps


def _host_finish(results, hidden, W, b, trans):
    """Strip-sum the device outputs, then levels 3..10 per core + the
    big-tree top 3 levels, in float64."""
    Texp = np.exp(trans.astype(np.float64)).reshape(L, L * L)   # [k, (l r)]
    W64 = W.astype(np.float64)
    b64 = b.astype(np.float64)

    score = np.zeros((N_CORES, 512, L))
    q = _bitrev(np.arange(512), 9)
    for c in range(N_CORES):
        lnS = results[c]["outResid"].astype(np.float64)         # [32, 512]
        heap1 = (1 << 12) - 1 + c * 512 + np.arange(512)
        E1 = hidden[heap1].astype(np.float64) @ W64.T + b64
        score[c] = E1 + lnS[:, q].T       # node j at col bitrev(j)

    for lev in range(2, 11):
        m = 1 << (10 - lev)
        d = DEPTH - lev
        left = score[:, 0::2]
        right = score[:, 1::2]
        heap = ((1 << d) - 1 + np.arange(N_CORES)[:, None] * m
                + np.arange(m)[None, :])                        # [8, m]
        Elev = hidden[heap].astype(np.float64) @ W64.T + b64
        ml = left.max(axis=2, keepdims=True)
        mr = right.max(axis=2, keepdims=True)
        P = (np.exp(left - ml)[..., :, None] *
             np.exp(right - mr)[..., None, :]).reshape(N_CORES, -1, L * L)
        score = Elev + np.log(P @ Texp.T) + ml + mr

    score = score.reshape(8, L)
    Etop = hidden[:7].astype(np.float64) @ W64.T + b64
    for d in (2, 1, 0):
        left = score[0::2]
        right = score[1::2]
        Elev = Etop[(1 << d) - 1: (1 << (d + 1)) - 1]
        ml = left.max(axis=1, keepdims=True)
        mr = right.max(axis=1, keepdims=True)
        P = (np.exp(left - ml)[:, :, None] *
             np.exp(right - mr)[:, None, :]).reshape(-1, L * L)
        score = Elev + np.log(P @ Texp.T) + ml + mr
    return score[0].astype(np.float32)


def _run_spmd(in_maps, trace=False):
    from concourse.bass_utils import run_bass_kernel_spmd
    nc = _build_bass()
    return run_bass_kernel_spmd(nc, in_maps, list(range(N_CORES)), trace=trace)


def kernel(hidden, W, b, trans):
    hidden = np.asarray(hidden, dtype=np.float32)
    W = np.asarray(W, dtype=np.float32)
    b = np.asarray(b, dtype=np.float32)
    trans = np.asarray(trans, dtype=np.float32)
    in_maps = _prep_in_maps(hidden, W, b, trans)
    res = _run_spmd(in_maps, trace=False)
    return _host_finish(res.results, hidden, W, b, trans)


# revision 59
# speedup vs baseline: 1.0133x; 1.0133x over previous
"""BinaryTreeCRF inside-algorithm kernel for 8 Trainium2 NeuronCores (v2).

Strategy (hardcoded for hidden=[16383,1024], L=32, depth 13):
  - Big tree cut at level 3: core c owns the subtree rooted at heap node 7+c.
  - Device does the bottom two combine levels (1024 leaves -> 512 -> 256
    nodes, 3 passes of 256 parents); host finishes levels 3..10 + the top
    3 big-tree levels in float64 (it also computes the emission rows for
    those 255+7 nodes itself, so their hidden rows never ship to devices).
  - Shipped columns (1792/core) are pass-major 256-col slices
    [rl0|rr0|elev0|rl1|rr1|elev1|elev2] so each combine pass depends on a
    prefix of the (FIFO-ordered) slice DMAs.
  - E is kept as 4 row-strips of partial sums ([128,n]: strip g rows hold
    chunks {g,g+4} of the K=1024 contraction) -> the E matmul uses all 128
    PE columns via 4x col-tiling; bias/4 is folded into each strip by the
    PSUM->SBUF copy. Strips are reduced for free inside the combine's
    replicate matmul (R1 selector).
  - Combine pass: stk2 = [rl;rr;rl;rr] actual values via 4 col-tiled
    replicate matmuls; logP_c = sel2_c^T @ stk (K=64, one MM per chunk,
    2x row-tiled); P = exp(logP) (ACT, no mean-centering needed in f32);
    S = texp^T @ P (8 accum MMs); resid = elev + ln S added in place into
    the elev strip block.
"""

import numpy as np
import ml_dtypes

BF16 = ml_dtypes.bfloat16

INPUT_SIZE = 1024
L = 32
DEPTH = 13
N_CORES = 8
NSLICE = 4
SL = 256
COLS = NSLICE * SL  # 1024


def _bitrev(x, bits):
    x = np.asarray(x, dtype=np.int64)
    out = np.zeros_like(x)
    for i in range(bits):
        out = (out << 1) | ((x >> i) & 1)
    return out


def _level_cols(c, lev):
    """Heap index per column q of the (bit-reversed) level-`lev` block."""
    m = 1 << (10 - lev)
    d = DEPTH - lev
    q = np.arange(m)
    j = _bitrev(q, 10 - lev)
    return (1 << d) - 1 + c * m + j


def _slices_for_core(c):
    lv0 = _level_cols(c, 0)
    return [lv0[0:256], lv0[512:768], lv0[256:512], lv0[768:1024]]


_NC = None


def _build_bass():
    global _NC
    if _NC is not None:
        return _NC
    from concourse import bacc, mybir
    from concourse.tile import TileContext
    from concourse.tile_rust import add_dep_helper

    dtb = mybir.dt.bfloat16
    dtf = mybir.dt.float32
    AF = mybir.ActivationFunctionType

    # Row-tiled logP (tile_position (64,0) K=64 matmuls) compiles and passes
    # CoreSim but faults on hardware — keep the flat variant.
    import os
    flat_logp = os.environ.get("V2_FLAT_LOGP", "1") == "1"

    dt8 = mybir.dt.float8e4

    nc = bacc.Bacc()
    hsB = nc.dram_tensor("hsB", [128, NSLICE, 8, SL], dt8, kind="ExternalInput")
    wpk = nc.dram_tensor("wpack", [128, 256], dt8, kind="ExternalInput")
    bpk = nc.dram_tensor("bpack", [128, 2], dtb, kind="ExternalInput")
    cpk = nc.dram_tensor("cpack2", [128, 1312], dtb, kind="ExternalInput")
    outR = nc.dram_tensor("outResid", [L, 2 * SL], dtb, kind="ExternalOutput")

    with TileContext(nc) as tc:
        with tc.tile_pool(name="consts", bufs=1) as consts, \
             tc.tile_pool(name="hs", bufs=1) as hpool, \
             tc.tile_pool(name="state", bufs=1) as state, \
             tc.tile_pool(name="stksb", bufs=2) as stksb, \
             tc.tile_pool(name="pbuf", bufs=2) as pbuf, \
             tc.tile_pool(name="tmp", bufs=4) as tmp, \
             tc.tile_pool(name="pslogp", bufs=1, space="PSUM") as pslogp, \
             tc.tile_pool(name="pse", bufs=2, space="PSUM") as pse, \
             tc.tile_pool(name="psstk", bufs=1, space="PSUM") as psstk, \
             tc.tile_pool(name="pss", bufs=1, space="PSUM") as pss:

            bp = consts.tile([128, 2], dtb, tag="bpack")
            nc.sync.dma_start(out=bp, in_=bpk[:, :])
            wp = consts.tile([128, 256], dt8, tag="wpack")
            wp_dma = nc.sync.dma_start(out=wp, in_=wpk[:, :])
            wp_dma = wp_dma.ins if hasattr(wp_dma, "ins") else wp_dma

            cp = consts.tile([128, 1312], dtb, tag="cpack2")
            cp_dma = nc.scalar.dma_start(out=cp, in_=cpk[:, :])
            cp_dma = cp_dma.ins if hasattr(cp_dma, "ins") else cp_dma

            R1_t = cp[:, 0:32]
            sel2_t = cp[:, 32:1056]      # chunk c at cols 32+128c; rows 0:64
                                         # (odd chunks also mirrored at 64:128)
            texp_t = cp[:, 1056:1312]

            # No inter-DMA deps: all on the one sync HWDGE ring, which
            # drains FIFO — serializing transfers without serializing the
            # ~2us completion receipts (dep-chaining cost ~3us per slice).
            hs_all = hpool.tile([128, NSLICE, 8, SL], dt8, tag="hs")
            for s in range(NSLICE):
                nc.sync.dma_start(out=hs_all[:, s, :, :], in_=hsB[:, s, :, :])

            E_bf = state.tile([128, COLS], dtb, tag="E_bf")

            # bias/4 per strip row, upcast to f32 (also triggers the ACT
            # table load early, off the critical path)
            bq_f = tmp.tile([128, 1], dtf, tag="bq_f")
            nc.scalar.activation(out=bq_f, in_=bp[:, 0:1], func=AF.Identity)

            # PE warm-up on a zeroed SBUF tile: no DMA dependency, so the
            # warm-up runs during the load phase (not after it, which would
            # push the first real matmul out by its whole duration) and the
            # HAM un-throttles before the first E matmul.
            jnkT = tmp.tile([128, SL], dtb, tag="jnkT")
            nc.gpsimd.memset(jnkT, 0.0)
            jnk = pse.tile([128, SL], dtf, tag="psE")
            for _ in range(14):
                nc.tensor.matmul(jnk, lhsT=jnkT[:, 0:128], rhs=jnkT,
                                 start=True, stop=True)

            def emit_E(s):
                psE = pse.tile([128, SL], dtf, tag="psE")
                for g in range(4):
                    for r in range(2):
                        ch = g + 4 * r
                        nc.tensor.matmul(
                            psE[32 * g:32 * g + 32, :],
                            lhsT=wp[:, 32 * ch:32 * ch + 32],
                            rhs=hs_all[:, s, ch, :],
                            start=(r == 0), stop=(r == 1),
                            tile_position=(0, 32 * g))
                # psE holds 16*E (W shipped as 16W for fp8 range): scale
                # back and add bias/4 in the one PSUM->SBUF copy
                nc.vector.tensor_scalar(
                    out=E_bf[:, s * SL:(s + 1) * SL], in0=psE,
                    scalar1=1.0 / 16.0, scalar2=bq_f,
                    op0=mybir.AluOpType.mult, op1=mybir.AluOpType.add)

            def combine_rep_logp(brl, brr):
                """Replicate+reduce strips, then the 8 K=64 logP matmuls.
                Full 256-parent passes: the logP matmuls are LDWEIGHTS-bound
                (P=128 selector loads), so N=256 keeps LDW:MM at 1:1."""
                stk2 = psstk.tile([128, SL], dtf, tag="stk2")
                for i, bsrc in enumerate((brl, brr, brl, brr)):
                    nc.tensor.matmul(
                        stk2[32 * i:32 * i + 32, :], lhsT=R1_t,
                        rhs=E_bf[:, bsrc:bsrc + SL],
                        start=True, stop=True, tile_position=(0, 32 * i))
                stk_sb = stksb.tile([128, SL], dtb, tag="stk_sb")
                nc.vector.tensor_copy(stk_sb, stk2)
                # Two separate PSUM tiles so each exp depends on only its 4
                # matmuls (Tile tracks deps at tile granularity): exp of the
                # first half overlaps the second half's matmuls.
                P = pbuf.tile([128, 8, SL], dtb, tag="P")
                for t, tag in ((0, "logP"), (1, "logPb")):
                    logP = pslogp.tile([128, 4, SL], dtf, tag=tag)
                    for c4 in range(4):
                        ch = 4 * t + c4
                        half = 0 if flat_logp else 64 * (ch & 1)
                        nc.tensor.matmul(
                            logP[:, c4, :],
                            lhsT=sel2_t[half:half + 64,
                                        128 * ch:128 * ch + 128],
                            rhs=stk_sb[half:half + 64, :],
                            start=True, stop=True, tile_position=(half, 0))
                    nc.scalar.activation(out=P[:, 4 * t:4 * t + 4, :],
                                         in_=logP, func=AF.Exp)
                return P

            lnS_out = state.tile([L, 2, SL], dtb, tag="lnS_out")

            def combine_S(P, pp):
                S = pss.tile([32, SL], dtf, tag="sps")
                for ch in range(8):
                    nc.tensor.matmul(
                        S, lhsT=texp_t[:, 32 * ch:32 * ch + 32],
                        rhs=P[:, ch, :], start=(ch == 0), stop=(ch == 7))
                nc.scalar.activation(out=lnS_out[:, pp, :], in_=S, func=AF.Ln)

            # pipeline: E slices chase the DMAs; combine passes chase E.
            # Device does only the leaf level and ships lnS: the host's f64
            # finisher adds its own emission rows for level 1 and up, so no
            # elev columns ever ship to the device and there is no resid-add
            # dependency on the kernel tail.
            emit_E(0)
            emit_E(1)
            P0 = combine_rep_logp(0, 256)
            emit_E(2)
            emit_E(3)
            combine_S(P0, 0)
            P1 = combine_rep_logp(512, 768)
            combine_S(P1, 1)

            nc.sync.dma_start(out=outR[:, :], in_=lnS_out)

    # Pin Exp/Ln/Identity to the one table set containing all three, so the
    # ACT engine loads its function table exactly once.
    import concourse.bacc as _bacc_mod
    from concourse.hw_specs import get_activation_tables as _gat
    _keep = "natural_log_exp_and_others"
    _pin = {AF.Exp, AF.Ln, AF.Identity, AF.Copy}

    def _gat_pinned(arch):
        t = _gat(arch)
        return {name: (funcs if name == _keep else (set(funcs) - _pin))
                for name, funcs in t.items()}

    _orig_gat = _bacc_mod.get_activation_tables
    _bacc_mod.get_activation_tables = _gat_pinned
    try:
        nc.compile()
    finally:
        _bacc_mod.get_activation_tables = _orig_gat
    _NC = nc
    return nc


def _patch_light_tail():
    """Sem-only end-of-kernel barriers (the default drain + two full
    all-engine barriers cost ~9us of kernel tail)."""
    from concourse import tile as _tile_mod
    from concourse.vector_clock import ScopedClock

    def _dab_light(self, tick_clock, wait_clock):
        drain_inst = self.nc.sync.drain()
        wait_clock.add_sem_waits(
            drain_inst.ins, ScopedClock({None: tick_clock.global_clock})
        )
        self.nc.all_engine_barrier(sem_only=True)
        popped = self.nc._tile_sem_poison_stack.pop()
        assert popped is self._sem_poison
        # No end-of-kernel sem clears: the Bass preamble range-clears every
        # kernel semaphore at the next launch, so the teardown is redundant.

    _tile_mod.TileContext._drain_and_barrier = _dab_light


_patch_light_tail()


def _prep_in_maps(hidden, W, b, trans):
    """Build per-core input dicts (host-side shard/transpose/cast)."""
    import ml_dtypes as _mld
    FP8 = _mld.float8_e4m3
    # wpack: E-matmul lhsT chunks, shipped as 16*W so the fp8 values sit in
    # e4m3's normal range (W ~ N(0, 1/32^2)); the device scales E back.
    wpack = np.zeros((128, 256), dtype=FP8)
    for ch in range(8):
        # wTr4[p, 32ch+m] = 16 * W[m, 128ch+p]
        wpack[:, 32 * ch:32 * ch + 32] = \
            (16.0 * W[:, 128 * ch:128 * ch + 128].T).astype(FP8)
    bpack = np.zeros((128, 2), dtype=BF16)
    bpack[:, 0] = np.tile(b.astype(np.float64) / 4.0, 4).astype(BF16)

    cpack2 = np.zeros((128, 1312), dtype=np.float32)
    for g in range(4):
        cpack2[32 * g:32 * g + 32, 0:32] = np.eye(32)
    for ch in range(8):
        blk = 32 + 128 * ch
        for p in range(128):
            # selector at rows 0:64 (all chunks) and mirrored at 64:128
            # (for the row-tiled variant's odd chunks)
            for rows in (0, 64):
                cpack2[rows + 4 * ch + p // 32, blk + p] += 1.0
                cpack2[rows + 32 + p % 32, blk + p] += 1.0
    Texp = np.exp(trans.astype(np.float64)).astype(np.float32)  # [k,l,r]
    for ch in range(8):
        for p in range(128):
            cpack2[p, 1056 + 32 * ch:1056 + 32 * ch + 32] = \
                Texp[:, 4 * ch + p // 32, p % 32]
    cpack2 = cpack2.astype(BF16)

    in_maps = []
    for c in range(N_CORES):
        cols = np.concatenate(_slices_for_core(c))
        hsc = hidden[cols].astype(FP8)                        # [1792, 1024]
        # hs[p, s, ch, j] = h[col s*256+j, 128ch+p]
        hs = np.ascontiguousarray(
            hsc.reshape(NSLICE, SL, 8, 128).transpose(3, 0, 2, 1))
        in_maps.append({"hsB": hs, "wpack": wpack, "bpack": bpack,
                        "cpack2": cpack2})
    return in_maps


def _host_finish(results, hidden, W, b, trans):
    """Strip-sum the device outputs, then levels 3..10 per core + the
    big-tree top 3 levels, in float64."""
    Texp = np.exp(trans.astype(np.float64)).reshape(L, L * L)   # [k, (l r)]
    W64 = W.astype(np.float64)
    b64 = b.astype(np.float64)

    score = np.zeros((N_CORES, 512, L))
    q = _bitrev(np.arange(512), 9)
    for c in range(N_CORES):
        lnS = results[c]["outResid"].astype(np.float64)         # [32, 512]
        heap1 = (1 << 12) - 1 + c * 512 + np.arange(512)
        E1 = hidden[heap1].astype(np.float64) @ W64.T + b64
        score[c] = E1 + lnS[:, q].T       # node j at col bitrev(j)

    for lev in range(2, 11):
        m = 1 << (10 - lev)
        d = DEPTH - lev
        left = score[:, 0::2]
        right = score[:, 1::2]
        heap = ((1 << d) - 1 + np.arange(N_CORES)[:, None] * m
                + np.arange(m)[None, :])                        # [8, m]
        Elev = hidden[heap].astype(np.float64) @ W64.T + b64
        ml = left.max(axis=2, keepdims=True)
        mr = right.max(axis=2, keepdims=True)
        P = (np.exp(left - ml)[..., :, None] *
             np.exp(right - mr)[..., None, :]).reshape(N_CORES, -1, L * L)
        score = Elev + np.log(P @ Texp.T) + ml + mr

    score = score.reshape(8, L)
    Etop = hidden[:7].astype(np.float64) @ W64.T + b64
    for d in (2, 1, 0):
        left = score[0::2]
        right = score[1::2]
        Elev = Etop[(1 << d) - 1: (1 << (d + 1)) - 1]
        ml = left.max(axis=1, keepdims=True)
        mr = right.max(axis=1, keepdims=True)
        P = (np.exp(left - ml)[:, :, None] *
             np.exp(right - mr)[:, None, :]).reshape(-1, L * L)
        score = Elev + np.log(P @ Texp.T) + ml + mr
    return score[0].astype(np.float32)


def _run_spmd(in_maps, trace=False):
    from concourse.bass_utils import run_bass_kernel_spmd
    nc = _build_bass()
    return run_bass_kernel_spmd(nc, in_maps, list(range(N_CORES)), trace=trace)


def kernel(hidden, W, b, trans):
    hidden = np.asarray(hidden, dtype=np.float32)
    W = np.asarray(W, dtype=np.float32)
    b = np.asarray(b, dtype=np.float32)
    trans = np.asarray(trans, dtype=np.float32)
    in_maps = _prep_in_maps(hidden, W, b, trans)
    res = _run_spmd(in_maps, trace=False)
    return _host_finish(res.results, hidden, W, b, trans)


# revision 60
# speedup vs baseline: 1.1173x; 1.1026x over previous
"""BinaryTreeCRF inside-algorithm kernel for 8 Trainium2 NeuronCores (v2).

Strategy (hardcoded for hidden=[16383,1024], L=32, depth 13):
  - Big tree cut at level 3: core c owns the subtree rooted at heap node 7+c.
  - Device does the bottom two combine levels (1024 leaves -> 512 -> 256
    nodes, 3 passes of 256 parents); host finishes levels 3..10 + the top
    3 big-tree levels in float64 (it also computes the emission rows for
    those 255+7 nodes itself, so their hidden rows never ship to devices).
  - Shipped columns (1792/core) are pass-major 256-col slices
    [rl0|rr0|elev0|rl1|rr1|elev1|elev2] so each combine pass depends on a
    prefix of the (FIFO-ordered) slice DMAs.
  - E is kept as 4 row-strips of partial sums ([128,n]: strip g rows hold
    chunks {g,g+4} of the K=1024 contraction) -> the E matmul uses all 128
    PE columns via 4x col-tiling; bias/4 is folded into each strip by the
    PSUM->SBUF copy. Strips are reduced for free inside the combine's
    replicate matmul (R1 selector).
  - Combine pass: stk2 = [rl;rr;rl;rr] actual values via 4 col-tiled
    replicate matmuls; logP_c = sel2_c^T @ stk (K=64, one MM per chunk,
    2x row-tiled); P = exp(logP) (ACT, no mean-centering needed in f32);
    S = texp^T @ P (8 accum MMs); resid = elev + ln S added in place into
    the elev strip block.
"""

import numpy as np
import ml_dtypes

BF16 = ml_dtypes.bfloat16

INPUT_SIZE = 1024
L = 32
DEPTH = 13
N_CORES = 8
NSLICE = 4
SL = 256
COLS = NSLICE * SL  # 1024


def _bitrev(x, bits):
    x = np.asarray(x, dtype=np.int64)
    out = np.zeros_like(x)
    for i in range(bits):
        out = (out << 1) | ((x >> i) & 1)
    return out


def _level_cols(c, lev):
    """Heap index per column q of the (bit-reversed) level-`lev` block."""
    m = 1 << (10 - lev)
    d = DEPTH - lev
    q = np.arange(m)
    j = _bitrev(q, 10 - lev)
    return (1 << d) - 1 + c * m + j


def _slices_for_core(c):
    lv0 = _level_cols(c, 0)
    return [lv0[0:256], lv0[512:768], lv0[256:512], lv0[768:1024]]


_NC = None


def _build_bass():
    global _NC
    if _NC is not None:
        return _NC
    from concourse import bacc, mybir
    from concourse.tile import TileContext
    from concourse.tile_rust import add_dep_helper

    dtb = mybir.dt.bfloat16
    dtf = mybir.dt.float32
    AF = mybir.ActivationFunctionType

    # Row-tiled logP (tile_position (64,0) K=64 matmuls) compiles and passes
    # CoreSim but faults on hardware — keep the flat variant.
    import os
    flat_logp = os.environ.get("V2_FLAT_LOGP", "1") == "1"

    dt8 = mybir.dt.float8e4

    nc = bacc.Bacc()
    hsB = nc.dram_tensor("hsB", [128, NSLICE, 8, SL], dt8, kind="ExternalInput")
    wpk = nc.dram_tensor("wpack", [128, 256], dt8, kind="ExternalInput")
    bpk = nc.dram_tensor("bpack", [128, 2], dtb, kind="ExternalInput")
    cpk = nc.dram_tensor("cpack2", [128, 1312], dtb, kind="ExternalInput")
    outR = nc.dram_tensor("outResid", [L, 2 * SL], dtb, kind="ExternalOutput")

    with TileContext(nc) as tc:
        with tc.tile_pool(name="consts", bufs=1) as consts, \
             tc.tile_pool(name="hs", bufs=1) as hpool, \
             tc.tile_pool(name="state", bufs=1) as state, \
             tc.tile_pool(name="stksb", bufs=2) as stksb, \
             tc.tile_pool(name="pbuf", bufs=2) as pbuf, \
             tc.tile_pool(name="tmp", bufs=4) as tmp, \
             tc.tile_pool(name="pslogp", bufs=1, space="PSUM") as pslogp, \
             tc.tile_pool(name="pse", bufs=2, space="PSUM") as pse, \
             tc.tile_pool(name="psstk", bufs=1, space="PSUM") as psstk, \
             tc.tile_pool(name="pss", bufs=1, space="PSUM") as pss:

            bp = consts.tile([128, 2], dtb, tag="bpack")
            nc.sync.dma_start(out=bp, in_=bpk[:, :])
            wp = consts.tile([128, 256], dt8, tag="wpack")
            wp_dma = nc.sync.dma_start(out=wp, in_=wpk[:, :])
            wp_dma = wp_dma.ins if hasattr(wp_dma, "ins") else wp_dma

            cp = consts.tile([128, 1312], dtb, tag="cpack2")
            cp_dma = nc.scalar.dma_start(out=cp, in_=cpk[:, :])
            cp_dma = cp_dma.ins if hasattr(cp_dma, "ins") else cp_dma

            R1_t = cp[:, 0:32]
            sel2_t = cp[:, 32:1056]      # chunk c at cols 32+128c; rows 0:64
                                         # (odd chunks also mirrored at 64:128)
            texp_t = cp[:, 1056:1312]

            # No inter-DMA deps: all on the one sync HWDGE ring, which
            # drains FIFO — serializing transfers without serializing the
            # ~2us completion receipts (dep-chaining cost ~3us per slice).
            hs_all = hpool.tile([128, NSLICE, 8, SL], dt8, tag="hs")
            for s in range(NSLICE):
                nc.sync.dma_start(out=hs_all[:, s, :, :], in_=hsB[:, s, :, :])

            E_bf = state.tile([128, COLS], dtb, tag="E_bf")

            # bias/4 per strip row, upcast to f32 (also triggers the ACT
            # table load early, off the critical path)
            bq_f = tmp.tile([128, 1], dtf, tag="bq_f")
            nc.scalar.activation(out=bq_f, in_=bp[:, 0:1], func=AF.Identity)

            # PE warm-up on a zeroed SBUF tile: no DMA dependency, so the
            # warm-up runs during the load phase (not after it, which would
            # push the first real matmul out by its whole duration) and the
            # HAM un-throttles before the first E matmul.
            jnkT = tmp.tile([128, SL], dtb, tag="jnkT")
            nc.gpsimd.memset(jnkT, 0.0)
            jnk = pse.tile([128, SL], dtf, tag="psE")
            # Sized to bridge the whole load phase (~5us: 16 cold + 16 warm
            # matmuls): ending early lets the PE idle just before the real
            # work and the HAM re-throttles it to 1.2 GHz for the combine.
            for _ in range(32):
                nc.tensor.matmul(jnk, lhsT=jnkT[:, 0:128], rhs=jnkT,
                                 start=True, stop=True)

            def emit_E(s):
                psE = pse.tile([128, SL], dtf, tag="psE")
                for g in range(4):
                    for r in range(2):
                        ch = g + 4 * r
                        nc.tensor.matmul(
                            psE[32 * g:32 * g + 32, :],
                            lhsT=wp[:, 32 * ch:32 * ch + 32],
                            rhs=hs_all[:, s, ch, :],
                            start=(r == 0), stop=(r == 1),
                            tile_position=(0, 32 * g))
                # psE holds 16*E (W shipped as 16W for fp8 range): scale
                # back and add bias/4 in the one PSUM->SBUF copy
                nc.vector.tensor_scalar(
                    out=E_bf[:, s * SL:(s + 1) * SL], in0=psE,
                    scalar1=1.0 / 16.0, scalar2=bq_f,
                    op0=mybir.AluOpType.mult, op1=mybir.AluOpType.add)

            def combine_rep_logp(brl, brr):
                """Replicate+reduce strips, then the 8 K=64 logP matmuls.
                Full 256-parent passes: the logP matmuls are LDWEIGHTS-bound
                (P=128 selector loads), so N=256 keeps LDW:MM at 1:1."""
                stk2 = psstk.tile([128, SL], dtf, tag="stk2")
                for i, bsrc in enumerate((brl, brr, brl, brr)):
                    nc.tensor.matmul(
                        stk2[32 * i:32 * i + 32, :], lhsT=R1_t,
                        rhs=E_bf[:, bsrc:bsrc + SL],
                        start=True, stop=True, tile_position=(0, 32 * i))
                stk_sb = stksb.tile([128, SL], dtb, tag="stk_sb")
                nc.vector.tensor_copy(stk_sb, stk2)
                # Two separate PSUM tiles so each exp depends on only its 4
                # matmuls (Tile tracks deps at tile granularity): exp of the
                # first half overlaps the second half's matmuls.
                P = pbuf.tile([128, 8, SL], dtb, tag="P")
                for t, tag in ((0, "logP"), (1, "logPb")):
                    logP = pslogp.tile([128, 4, SL], dtf, tag=tag)
                    for c4 in range(4):
                        ch = 4 * t + c4
                        half = 0 if flat_logp else 64 * (ch & 1)
                        nc.tensor.matmul(
                            logP[:, c4, :],
                            lhsT=sel2_t[half:half + 64,
                                        128 * ch:128 * ch + 128],
                            rhs=stk_sb[half:half + 64, :],
                            start=True, stop=True, tile_position=(half, 0))
                    nc.scalar.activation(out=P[:, 4 * t:4 * t + 4, :],
                                         in_=logP, func=AF.Exp)
                return P

            lnS_out = state.tile([L, 2, SL], dtb, tag="lnS_out")

            def combine_S(P, pp):
                S = pss.tile([32, SL], dtf, tag="sps")
                for ch in range(8):
                    nc.tensor.matmul(
                        S, lhsT=texp_t[:, 32 * ch:32 * ch + 32],
                        rhs=P[:, ch, :], start=(ch == 0), stop=(ch == 7))
                nc.scalar.activation(out=lnS_out[:, pp, :], in_=S, func=AF.Ln)

            # pipeline: E slices chase the DMAs; combine passes chase E.
            # Device does only the leaf level and ships lnS: the host's f64
            # finisher adds its own emission rows for level 1 and up, so no
            # elev columns ever ship to the device and there is no resid-add
            # dependency on the kernel tail.
            emit_E(0)
            emit_E(1)
            P0 = combine_rep_logp(0, 256)
            emit_E(2)
            emit_E(3)
            combine_S(P0, 0)
            P1 = combine_rep_logp(512, 768)
            combine_S(P1, 1)

            nc.sync.dma_start(out=outR[:, :], in_=lnS_out)

    # Pin Exp/Ln/Identity to the one table set containing all three, so the
    # ACT engine loads its function table exactly once.
    import concourse.bacc as _bacc_mod
    from concourse.hw_specs import get_activation_tables as _gat
    _keep = "natural_log_exp_and_others"
    _pin = {AF.Exp, AF.Ln, AF.Identity, AF.Copy}

    def _gat_pinned(arch):
        t = _gat(arch)
        return {name: (funcs if name == _keep else (set(funcs) - _pin))
                for name, funcs in t.items()}

    _orig_gat = _bacc_mod.get_activation_tables
    _bacc_mod.get_activation_tables = _gat_pinned
    try:
        nc.compile()
    finally:
        _bacc_mod.get_activation_tables = _orig_gat
    _NC = nc
    return nc


def _patch_light_tail():
    """Sem-only end-of-kernel barriers (the default drain + two full
    all-engine barriers cost ~9us of kernel tail)."""
    from concourse import tile as _tile_mod
    from concourse.vector_clock import ScopedClock

    def _dab_light(self, tick_clock, wait_clock):
        drain_inst = self.nc.sync.drain()
        wait_clock.add_sem_waits(
            drain_inst.ins, ScopedClock({None: tick_clock.global_clock})
        )
        self.nc.all_engine_barrier(sem_only=True)
        popped = self.nc._tile_sem_poison_stack.pop()
        assert popped is self._sem_poison
        # No end-of-kernel sem clears: the Bass preamble range-clears every
        # kernel semaphore at the next launch, so the teardown is redundant.

    _tile_mod.TileContext._drain_and_barrier = _dab_light


_patch_light_tail()


def _prep_in_maps(hidden, W, b, trans):
    """Build per-core input dicts (host-side shard/transpose/cast)."""
    import ml_dtypes as _mld
    FP8 = _mld.float8_e4m3
    # wpack: E-matmul lhsT chunks, shipped as 16*W so the fp8 values sit in
    # e4m3's normal range (W ~ N(0, 1/32^2)); the device scales E back.
    wpack = np.zeros((128, 256), dtype=FP8)
    for ch in range(8):
        # wTr4[p, 32ch+m] = 16 * W[m, 128ch+p]
        wpack[:, 32 * ch:32 * ch + 32] = \
            (16.0 * W[:, 128 * ch:128 * ch + 128].T).astype(FP8)
    bpack = np.zeros((128, 2), dtype=BF16)
    bpack[:, 0] = np.tile(b.astype(np.float64) / 4.0, 4).astype(BF16)

    cpack2 = np.zeros((128, 1312), dtype=np.float32)
    for g in range(4):
        cpack2[32 * g:32 * g + 32, 0:32] = np.eye(32)
    for ch in range(8):
        blk = 32 + 128 * ch
        for p in range(128):
            # selector at rows 0:64 (all chunks) and mirrored at 64:128
            # (for the row-tiled variant's odd chunks)
            for rows in (0, 64):
                cpack2[rows + 4 * ch + p // 32, blk + p] += 1.0
                cpack2[rows + 32 + p % 32, blk + p] += 1.0
    Texp = np.exp(trans.astype(np.float64)).astype(np.float32)  # [k,l,r]
    for ch in range(8):
        for p in range(128):
            cpack2[p, 1056 + 32 * ch:1056 + 32 * ch + 32] = \
                Texp[:, 4 * ch + p // 32, p % 32]
    cpack2 = cpack2.astype(BF16)

    in_maps = []
    for c in range(N_CORES):
        cols = np.concatenate(_slices_for_core(c))
        hsc = hidden[cols].astype(FP8)                        # [1792, 1024]
        # hs[p, s, ch, j] = h[col s*256+j, 128ch+p]
        hs = np.ascontiguousarray(
            hsc.reshape(NSLICE, SL, 8, 128).transpose(3, 0, 2, 1))
        in_maps.append({"hsB": hs, "wpack": wpack, "bpack": bpack,
                        "cpack2": cpack2})
    return in_maps


def _host_finish(results, hidden, W, b, trans):
    """Strip-sum the device outputs, then levels 3..10 per core + the
    big-tree top 3 levels, in float64."""
    Texp = np.exp(trans.astype(np.float64)).reshape(L, L * L)   # [k, (l r)]
    W64 = W.astype(np.float64)
    b64 = b.astype(np.float64)

    score = np.zeros((N_CORES, 512, L))
    q = _bitrev(np.arange(512), 9)
    for c in range(N_CORES):
        lnS = results[c]["outResid"].astype(np.float64)         # [32, 512]
        heap1 = (1 << 12) - 1 + c * 512 + np.arange(512)
        E1 = hidden[heap1].astype(np.float64) @ W64.T + b64
        score[c] = E1 + lnS[:, q].T       # node j at col bitrev(j)

    for lev in range(2, 11):
        m = 1 << (10 - lev)
        d = DEPTH - lev
        left = score[:, 0::2]
        right = score[:, 1::2]
        heap = ((1 << d) - 1 + np.arange(N_CORES)[:, None] * m
                + np.arange(m)[None, :])                        # [8, m]
        Elev = hidden[heap].astype(np.float64) @ W64.T + b64
        ml = left.max(axis=2, keepdims=True)
        mr = right.max(axis=2, keepdims=True)
        P = (np.exp(left - ml)[..., :, None] *
             np.exp(right - mr)[..., None, :]).reshape(N_CORES, -1, L * L)
        score = Elev + np.log(P @ Texp.T) + ml + mr

    score = score.reshape(8, L)
    Etop = hidden[:7].astype(np.float64) @ W64.T + b64
    for d in (2, 1, 0):
        left = score[0::2]
        right = score[1::2]
        Elev = Etop[(1 << d) - 1: (1 << (d + 1)) - 1]
        ml = left.max(axis=1, keepdims=True)
        mr = right.max(axis=1, keepdims=True)
        P = (np.exp(left - ml)[:, :, None] *
             np.exp(right - mr)[:, None, :]).reshape(-1, L * L)
        score = Elev + np.log(P @ Texp.T) + ml + mr
    return score[0].astype(np.float32)


def _run_spmd(in_maps, trace=False):
    from concourse.bass_utils import run_bass_kernel_spmd
    nc = _build_bass()
    return run_bass_kernel_spmd(nc, in_maps, list(range(N_CORES)), trace=trace)


def kernel(hidden, W, b, trans):
    hidden = np.asarray(hidden, dtype=np.float32)
    W = np.asarray(W, dtype=np.float32)
    b = np.asarray(b, dtype=np.float32)
    trans = np.asarray(trans, dtype=np.float32)
    in_maps = _prep_in_maps(hidden, W, b, trans)
    res = _run_spmd(in_maps, trace=False)
    return _host_finish(res.results, hidden, W, b, trans)
